# revision 1
# baseline (speedup 1.0000x reference)
"""Multi-head attention (B=2, S=2048, D=1024, H=16) on 8 NeuronCores.

Sharding: Megatron tensor parallelism. Core r owns heads 2r, 2r+1
(a 128-wide slice of D). Wq/Wk/Wv column-parallel, Wo row-parallel,
ReduceScatter(add) over tokens at the end; host concatenates the 8
token slices and adds bo.

Layouts on device (per core):
  xqT/xkT/xvT : [1024, 4096]  host-transposed activations (feature-major)
  qT/kT       : [128, 2048]   per batch, dk-major (rows = this core's 2 heads)
  v           : [128, 130]    16 token-tiles per batch; cols = [v_h0 | 1 | v_h1 | 1]
                              (ones column makes the PV matmul emit softmax sums)
  scores^T    : psum [128 sk, 512 sq] -> exp on ACT -> PT sbuf
  PV          : psum [65, 512] accumulated over 16 sk tiles; row 64 = sums
  attnT       : [128, 2048]   per batch, normalized, = lhsT for Wo matmul
"""

import sys

sys.path.insert(0, "/opt/trn_rl_repo")

import numpy as np

B, S, D, H, DK = 2, 2048, 1024, 16, 64
NCORES = 8
TOK = B * S            # 4096
DKC = D // NCORES      # 128 = 2 heads per core
TOKC = TOK // NCORES   # 512 output rows per core
KT = D // 128          # 8 contraction tiles
SKT = S // 128         # 16 key tiles per batch
SQB = S // 512         # 4 query blocks per batch

# matmul operand dtype: float32 (exact, 4 cyc/row) or float32r (1 cyc/row)
MM_DT_NAME = "float32r"

_cache = {}


def _build(collective=True):
    from contextlib import ExitStack

    from concourse import bacc
    import concourse.mybir as mybir
    import concourse.tile as tile

    f32 = mybir.dt.float32
    mm_dt = getattr(mybir.dt, MM_DT_NAME)
    Act = mybir.ActivationFunctionType

    def c(ap):
        # bitcast DRAM sources feeding matmul-operand tiles to the matmul dtype
        return ap.bitcast(mm_dt) if mm_dt != f32 else ap

    nc = bacc.Bacc(
        "TRN2", target_bir_lowering=False, debug=False,
        enable_asserts=False, num_devices=NCORES,
    )

    xqT = nc.dram_tensor("xqT", [D, TOK], f32, kind="ExternalInput").ap()
    xkT = nc.dram_tensor("xkT", [D, TOK], f32, kind="ExternalInput").ap()
    xvT = nc.dram_tensor("xvT", [D, TOK], f32, kind="ExternalInput").ap()
    wq = nc.dram_tensor("wq", [D, DKC], f32, kind="ExternalInput").ap()
    wk = nc.dram_tensor("wk", [D, DKC], f32, kind="ExternalInput").ap()
    wv = nc.dram_tensor("wv", [D, DKC], f32, kind="ExternalInput").ap()
    wo = nc.dram_tensor("wo", [DKC, D], f32, kind="ExternalInput").ap()
    bq = nc.dram_tensor("bq", [DKC, 1], f32, kind="ExternalInput").ap()
    bk = nc.dram_tensor("bk", [DKC, 1], f32, kind="ExternalInput").ap()
    bv = nc.dram_tensor("bv", [1, DKC], f32, kind="ExternalInput").ap()
    out_ext = nc.dram_tensor("out", [TOKC, D], f32, kind="ExternalOutput").ap()

    with tile.TileContext(nc) as tc, ExitStack() as ctx, \
            nc.allow_low_precision("float32r matmul operands, fp32 psum accumulate"):
        wpool = ctx.enter_context(tc.tile_pool(name="w", bufs=1))
        xpool = ctx.enter_context(tc.tile_pool(name="x", bufs=12))
        qkpool = ctx.enter_context(tc.tile_pool(name="qk", bufs=2))
        vpool = ctx.enter_context(tc.tile_pool(name="v", bufs=32))
        ptpool = ctx.enter_context(tc.tile_pool(name="pt", bufs=6))
        atpool = ctx.enter_context(tc.tile_pool(name="at", bufs=2))
        smpool = ctx.enter_context(tc.tile_pool(name="sm", bufs=4))
        opool = ctx.enter_context(tc.tile_pool(name="o", bufs=4))
        ps_mm = ctx.enter_context(tc.tile_pool(name="psmm", bufs=3, space="PSUM"))
        ps_acc = ctx.enter_context(tc.tile_pool(name="psacc", bufs=2, space="PSUM"))
        dram = ctx.enter_context(tc.tile_pool(name="dram", bufs=1, space="DRAM"))

        # ---- constants / weights into SBUF ----
        wq_t, wk_t, wv_t = [], [], []
        for name, src, lst in (("wq", wq, wq_t), ("wk", wk, wk_t), ("wv", wv, wv_t)):
            for k in range(KT):
                t = wpool.tile([128, DKC], mm_dt, tag=f"{name}{k}")
                nc.sync.dma_start(t[:], c(src[k * 128:(k + 1) * 128, :]))
                lst.append(t)
        wo_t = wpool.tile([DKC, D], mm_dt, tag="wo")
        nc.sync.dma_start(wo_t[:], c(wo[:]))
        bq_t = wpool.tile([DKC, 1], f32, tag="bq")
        nc.sync.dma_start(bq_t[:], bq[:])
        bk_t = wpool.tile([DKC, 1], f32, tag="bk")
        nc.sync.dma_start(bk_t[:], bk[:])
        bv_t = wpool.tile([1, DKC], mm_dt, tag="bv")
        nc.sync.dma_start(bv_t[:], c(bv[:]))
        ones_f = wpool.tile([1, 128], f32, tag="onesf")
        nc.gpsimd.memset(ones_f[:], 1.0)
        ones_t = wpool.tile([1, 128], mm_dt, tag="ones")
        nc.vector.tensor_copy(ones_t[:], ones_f[:])
        onescol_f = wpool.tile([128, 1], f32, tag="onescolf")
        nc.gpsimd.memset(onescol_f[:], 1.0)

        partial = dram.tile([TOK, D], f32, tag="partial")
        rs_out = dram.tile([TOKC, D], f32, tag="rsout")

        for b in range(B):
            t0 = b * S
            # ---- q/k projections -> qT_b, kT_b [128, S] (dk-major) ----
            qT_b = qkpool.tile([128, S], mm_dt, tag="qT")
            kT_b = qkpool.tile([128, S], mm_dt, tag="kT")
            for xT, w_list, bias_t, dst in (
                (xqT, wq_t, bq_t, qT_b), (xkT, wk_t, bk_t, kT_b),
            ):
                for blk in range(SQB):
                    ps = ps_mm.tile([128, 512], f32, tag="mm")
                    for k in range(KT):
                        xt = xpool.tile([128, 512], mm_dt, tag="xt")
                        nc.sync.dma_start(
                            xt[:],
                            c(xT[k * 128:(k + 1) * 128,
                                 t0 + blk * 512: t0 + (blk + 1) * 512]),
                        )
                        nc.tensor.matmul(
                            ps[:], lhsT=w_list[k][:], rhs=xt[:],
                            start=(k == 0), stop=(k == KT - 1),
                        )
                    nc.scalar.activation(
                        dst[:, blk * 512:(blk + 1) * 512], ps[:],
                        Act.Identity, bias=bias_t[:, 0:1],
                    )

            # ---- v projection -> 16 tiles [128 tok, 130] ----
            v_tiles = []
            for blk in range(SQB):
                xv_blk = []
                for k in range(KT):
                    xt = xpool.tile([128, 512], mm_dt, tag="xt")
                    nc.sync.dma_start(
                        xt[:],
                        c(xvT[k * 128:(k + 1) * 128,
                              t0 + blk * 512: t0 + (blk + 1) * 512]),
                    )
                    xv_blk.append(xt)
                for mi in range(4):
                    ps = ps_mm.tile([128, DKC], f32, tag="mm")
                    for k in range(KT):
                        nc.tensor.matmul(
                            ps[:], lhsT=xv_blk[k][:, mi * 128:(mi + 1) * 128],
                            rhs=wv_t[k][:], start=(k == 0), stop=False,
                        )
                    nc.tensor.matmul(
                        ps[:], lhsT=ones_t[0:1, :], rhs=bv_t[:],
                        start=False, stop=True,
                    )
                    vt = vpool.tile([128, 130], mm_dt, tag="v")
                    nc.vector.tensor_copy(vt[:, 0:64], ps[:, 0:64])
                    nc.vector.tensor_copy(vt[:, 65:129], ps[:, 64:128])
                    nc.vector.tensor_copy(vt[:, 64:65], onescol_f[:])
                    nc.vector.tensor_copy(vt[:, 129:130], onescol_f[:])
                    v_tiles.append(vt)

            # ---- attention (2 heads) -> attnT_b [128, S] ----
            attnT_b = atpool.tile([128, S], mm_dt, tag="attnT")
            for h in range(2):
                hp = h * 64
                for sq in range(SQB):
                    qs = slice(sq * 512, (sq + 1) * 512)
                    xps = ps_acc.tile([65, 512], f32, tag="acc")
                    for sk in range(SKT):
                        sps = ps_mm.tile([128, 512], f32, tag="mm")
                        nc.tensor.matmul(
                            sps[:],
                            lhsT=kT_b[hp:hp + 64, sk * 128:(sk + 1) * 128],
                            rhs=qT_b[hp:hp + 64, qs],
                            start=True, stop=True,
                        )
                        pt = ptpool.tile([128, 512], mm_dt, tag="pt")
                        nc.scalar.activation(pt[:], sps[:], Act.Exp, scale=0.125)
                        nc.tensor.matmul(
                            xps[:], lhsT=v_tiles[sk][:, h * 65:h * 65 + 65],
                            rhs=pt[:], start=(sk == 0), stop=(sk == SKT - 1),
                        )
                    rec = smpool.tile([1, 512], mm_dt, tag="rec")
                    nc.vector.reciprocal(rec[:], xps[64:65, :])
                    rbp = ps_mm.tile([64, 512], f32, tag="mm")
                    nc.tensor.matmul(
                        rbp[:], lhsT=ones_t[0:1, 0:64], rhs=rec[:],
                        start=True, stop=True,
                    )
                    rb = smpool.tile([64, 512], f32, tag="rb")
                    nc.scalar.copy(rb[:], rbp[:])
                    nc.vector.tensor_mul(
                        attnT_b[hp:hp + 64, qs], xps[0:64, :], rb[:],
                    )

            # ---- output projection partial [S, D] ----
            for m in range(S // 128):
                for n2 in range(2):
                    ops = ps_mm.tile([128, 512], f32, tag="mm")
                    nc.tensor.matmul(
                        ops[:], lhsT=attnT_b[:, m * 128:(m + 1) * 128],
                        rhs=wo_t[:, n2 * 512:(n2 + 1) * 512],
                        start=True, stop=True,
                    )
                    ot = opool.tile([128, 512], f32, tag="ot")
                    nc.vector.tensor_copy(ot[:], ops[:])
                    nc.sync.dma_start(
                        partial[t0 + m * 128: t0 + (m + 1) * 128,
                                n2 * 512:(n2 + 1) * 512],
                        ot[:],
                    )

        if collective:
            nc.gpsimd.collective_compute(
                "ReduceScatter",
                mybir.AluOpType.add,
                replica_groups=[list(range(NCORES))],
                ins=[partial.opt()],
                outs=[rs_out.opt()],
            )
            nc.sync.dma_start(out_ext[:], rs_out[:])
        else:
            nc.sync.dma_start(out_ext[:], partial[0:TOKC, :])

    nc.compile()
    return nc


def _get_nc():
    if "nc" not in _cache:
        _cache["nc"] = _build()
    return _cache["nc"]


def kernel(query, key, value, Wq, bq, Wk, bk, Wv, bv, Wo, bo, trace=False):
    from concourse.bass_utils import run_bass_kernel_spmd

    nc = _get_nc()

    q = np.ascontiguousarray(np.asarray(query, np.float32).reshape(TOK, D).T)
    k = np.ascontiguousarray(np.asarray(key, np.float32).reshape(TOK, D).T)
    v = np.ascontiguousarray(np.asarray(value, np.float32).reshape(TOK, D).T)
    Wq = np.asarray(Wq, np.float32)
    Wk = np.asarray(Wk, np.float32)
    Wv = np.asarray(Wv, np.float32)
    Wo = np.asarray(Wo, np.float32)

    in_maps = []
    for r in range(NCORES):
        sl = slice(r * DKC, (r + 1) * DKC)
        in_maps.append({
            "xqT": q, "xkT": k, "xvT": v,
            "wq": np.ascontiguousarray(Wq[:, sl]),
            "wk": np.ascontiguousarray(Wk[:, sl]),
            "wv": np.ascontiguousarray(Wv[:, sl]),
            "wo": np.ascontiguousarray(Wo[sl, :]),
            "bq": np.ascontiguousarray(np.asarray(bq, np.float32)[sl, None]),
            "bk": np.ascontiguousarray(np.asarray(bk, np.float32)[sl, None]),
            "bv": np.ascontiguousarray(np.asarray(bv, np.float32)[None, sl]),
        })

    res = run_bass_kernel_spmd(nc, in_maps, list(range(NCORES)), trace=trace)
    _cache["last_results"] = res

    out = np.concatenate([res.results[r]["out"] for r in range(NCORES)], axis=0)
    out = out + np.asarray(bo, np.float32)[None, :]
    return out.reshape(B, S, D)



# revision 6
# speedup vs baseline: 2.6484x; 2.6484x over previous
"""Multi-head attention (B=2, S=2048, D=1024, H=16) on 8 NeuronCores.

Sharding: DP(batch) x TP(heads). Core r handles batch r//4 and heads
[4*(r%4), 4*(r%4)+4) as two head-pairs. Per core:
  - q/k/v projections for its batch tokens x its 256 dk columns
  - attention for its 4 heads (head-pair packed matmuls:
      scores: two K=64 matmuls row-packed at partitions 0/64
      PV:     two M=64 matmuls col-packed at psum rows 0/64
      denom:  M=1 ones-matmuls col-packed at psum rows 0/32/64/96)
  - Wo row-slice partial [2048, 1024]
No collectives: the host sums the 4 partials per batch and adds bo.

All matmul operands are bf16 (fp32 PSUM accumulate). exp on ACT as one
[128,1024] instruction spanning 2 psum banks per (pair, sq, sk).
Softmax 1/den via reciprocal_approx_fast batched [97,512].
"""

import sys

sys.path.insert(0, "/opt/trn_rl_repo")

import numpy as np

B, S, D, H, DK = 2, 2048, 1024, 16, 64
NCORES = 8
GC = 4                 # cores per batch group
HPC = 4                # heads per core
DKC = HPC * DK         # 256 dk columns per core
NPAIR = 2              # head pairs per core
KT = D // 128          # 8 contraction tiles for projections
SQB = S // 512         # 4 query blocks
SKT = S // 128         # 16 key tiles

_cache = {}


def _build():
    from contextlib import ExitStack

    from concourse import bacc
    import concourse.mybir as mybir
    import concourse.tile as tile

    f32 = mybir.dt.float32
    bf16 = mybir.dt.bfloat16
    Act = mybir.ActivationFunctionType

    nc = bacc.Bacc(
        "TRN2", target_bir_lowering=False, debug=False,
        enable_asserts=False, num_devices=NCORES,
    )

    xqT = nc.dram_tensor("xqT", [D, S], bf16, kind="ExternalInput").ap()
    xkT = nc.dram_tensor("xkT", [D, S], bf16, kind="ExternalInput").ap()
    xvT = nc.dram_tensor("xvT", [D, S], bf16, kind="ExternalInput").ap()
    wq = nc.dram_tensor("wq", [D, DKC], bf16, kind="ExternalInput").ap()
    wk = nc.dram_tensor("wk", [D, DKC], bf16, kind="ExternalInput").ap()
    wv = nc.dram_tensor("wv", [D, DKC], bf16, kind="ExternalInput").ap()
    wo = nc.dram_tensor("wo", [DKC, D], bf16, kind="ExternalInput").ap()
    bq = nc.dram_tensor("bq", [DKC, 1], f32, kind="ExternalInput").ap()
    bk = nc.dram_tensor("bk", [DKC, 1], f32, kind="ExternalInput").ap()
    bv = nc.dram_tensor("bv", [1, DKC], bf16, kind="ExternalInput").ap()
    out_ext = nc.dram_tensor("out", [S, D], bf16, kind="ExternalOutput").ap()

    with tile.TileContext(nc) as tc, ExitStack() as ctx, \
            nc.allow_low_precision("bf16 matmul operands, fp32 psum accumulate"):
        wpool = ctx.enter_context(tc.tile_pool(name="w", bufs=1))
        xpool = ctx.enter_context(tc.tile_pool(name="x", bufs=16))
        qkpool = ctx.enter_context(tc.tile_pool(name="qk", bufs=1))
        vpool = ctx.enter_context(tc.tile_pool(name="v", bufs=16))
        ptpool = ctx.enter_context(tc.tile_pool(name="pt", bufs=4))
        atpool = ctx.enter_context(tc.tile_pool(name="at", bufs=1))
        smpool = ctx.enter_context(tc.tile_pool(name="sm", bufs=2))
        pvsb = ctx.enter_context(tc.tile_pool(name="pvsb", bufs=4))
        opool = ctx.enter_context(tc.tile_pool(name="o", bufs=3))
        ps_sc = ctx.enter_context(tc.tile_pool(name="pssc", bufs=2, space="PSUM"))
        ps_pv = ctx.enter_context(tc.tile_pool(name="pspv", bufs=2, space="PSUM"))
        ps_dn = ctx.enter_context(tc.tile_pool(name="psdn", bufs=1, space="PSUM"))
        ps_sh = ctx.enter_context(tc.tile_pool(name="pssh", bufs=1, space="PSUM"))

        # ---- weights / constants into SBUF ----
        wq_t, wk_t, wv_t = [], [], []
        for name, src, lst in (("wq", wq, wq_t), ("wk", wk, wk_t), ("wv", wv, wv_t)):
            for k in range(KT):
                t = wpool.tile([128, DKC], bf16, tag=f"{name}{k}", name=f"{name}{k}")
                nc.sync.dma_start(t[:], src[k * 128:(k + 1) * 128, :])
                lst.append(t)
        wo_t = []
        for k in range(2):
            t = wpool.tile([128, D], bf16, tag=f"wo{k}", name=f"wo{k}")
            nc.sync.dma_start(t[:], wo[k * 128:(k + 1) * 128, :])
            wo_t.append(t)
        bq_t, bk_t = [], []
        for p in range(NPAIR):
            t = wpool.tile([128, 1], f32, tag=f"bq{p}", name=f"bq{p}")
            nc.sync.dma_start(t[:], bq[p * 128:(p + 1) * 128, :])
            bq_t.append(t)
            t = wpool.tile([128, 1], f32, tag=f"bk{p}", name=f"bk{p}")
            nc.sync.dma_start(t[:], bk[p * 128:(p + 1) * 128, :])
            bk_t.append(t)
        bv_t = wpool.tile([1, DKC], bf16, tag="bv")
        nc.sync.dma_start(bv_t[:], bv[:])

        onesf = wpool.tile([128, 128], f32, tag="onesf")
        nc.gpsimd.memset(onesf[:], 1.0)
        ones_col = wpool.tile([128, 1], bf16, tag="onescol")
        nc.vector.tensor_copy(ones_col[:], onesf[:, 0:1])
        ones_row = wpool.tile([1, 128], bf16, tag="onesrow")
        nc.vector.tensor_copy(ones_row[:], onesf[0:1, :])
        # E97 selectors: rb rows 0:64 <- rec row 64p, rows 64:128 <- row 64p+32
        e97 = []
        for p in range(NPAIR):
            ef = wpool.tile([97, 128], f32, tag=f"e97f{p}", name=f"e97f{p}")
            nc.gpsimd.memset(ef[:], 0.0)
            nc.gpsimd.memset(ef[64 * p:64 * p + 1, 0:64], 1.0)
            nc.gpsimd.memset(ef[64 * p + 32:64 * p + 33, 64:128], 1.0)
            eb = wpool.tile([97, 128], bf16, tag=f"e97_{p}", name=f"e97_{p}")
            nc.vector.tensor_copy(eb[:], ef[:])
            e97.append(eb)

        # ---- k/q projections -> kT_p, qT_p [128, S] bf16 ----
        qT = [qkpool.tile([128, S], bf16, tag=f"qT{p}", name=f"qT{p}")
              for p in range(NPAIR)]
        kTt = [qkpool.tile([128, S], bf16, tag=f"kT{p}", name=f"kT{p}")
               for p in range(NPAIR)]
        for xT, w_list, b_list, dsts in (
            (xkT, wk_t, bk_t, kTt), (xqT, wq_t, bq_t, qT),
        ):
            for blk in range(SQB):
                xts = []
                for k in range(KT):
                    xt = xpool.tile([128, 512], bf16, tag="xt")
                    nc.sync.dma_start(
                        xt[:], xT[k * 128:(k + 1) * 128, blk * 512:(blk + 1) * 512])
                    xts.append(xt)
                for p, pspool in ((0, ps_pv), (1, ps_dn)):
                    ps = pspool.tile([128, 512], f32, tag="mm", name="projps")
                    for k in range(KT):
                        nc.tensor.matmul(
                            ps[:], lhsT=w_list[k][:, p * 128:(p + 1) * 128],
                            rhs=xts[k][:], start=(k == 0), stop=(k == KT - 1),
                        )
                    nc.scalar.activation(
                        dsts[p][:, blk * 512:(blk + 1) * 512], ps[:],
                        Act.Identity, bias=b_list[p][:, 0:1],
                    )

        # ---- v projection -> 16 tiles [128 tok, 256] bf16 ----
        v_t = []
        for blk in range(SQB):
            xts = []
            for k in range(KT):
                xt = xpool.tile([128, 512], bf16, tag="xt")
                nc.sync.dma_start(
                    xt[:], xvT[k * 128:(k + 1) * 128, blk * 512:(blk + 1) * 512])
                xts.append(xt)
            for mi in range(4):
                pspool, pstag = ((ps_pv, "mm"), (ps_dn, "mm"),
                                 (ps_sh, "sh"), (ps_pv, "mm"))[mi]
                ps = pspool.tile([128, DKC], f32, tag=pstag, name="vps")
                for k in range(KT):
                    nc.tensor.matmul(
                        ps[:, 0:DKC], lhsT=xts[k][:, mi * 128:(mi + 1) * 128],
                        rhs=wv_t[k][:], start=(k == 0), stop=False,
                    )
                nc.tensor.matmul(
                    ps[:, 0:DKC], lhsT=ones_row[0:1, :], rhs=bv_t[:],
                    start=False, stop=True,
                )
                vt = vpool.tile([128, DKC], bf16, tag="v")
                nc.vector.tensor_copy(vt[:], ps[:, 0:DKC])
                v_t.append(vt)

        # ---- attention + Wo ----
        attnT = [atpool.tile([128, S], bf16, tag=f"attnT{p}", name=f"attnT{p}")
                 for p in range(NPAIR)]
        wo_pending = []    # (m, n) groups whose attnT deps are satisfied
        norm_pending = []  # deferred normalization closures

        def emit_wo(ms, pspool, pstag):
            for m in ms:
                ot = opool.tile([128, D], bf16, tag="ot", name="ot")
                for n in range(2):
                    ps = pspool.tile([128, 512], f32, tag=pstag, name="wops")
                    for p in range(NPAIR):
                        nc.tensor.matmul(
                            ps[:], lhsT=attnT[p][:, m * 128:(m + 1) * 128],
                            rhs=wo_t[p][:, n * 512:(n + 1) * 512],
                            start=(p == 0), stop=(p == NPAIR - 1),
                        )
                    nc.vector.tensor_copy(ot[:, n * 512:(n + 1) * 512], ps[:])
                nc.sync.dma_start(out_ext[m * 128:(m + 1) * 128, :], ot[:])

        def do_norm(p, sq, pv_sb, rec_b):
            # rb = broadcast of rec rows (64p, 64p+32) over dk rows; then
            # attnT[:, sq block] = pv_sb * rb
            rbp = ps_sh.tile([128, 512], f32, tag="sh", name="rbp")
            nc.tensor.matmul(rbp[:], lhsT=e97[p][:], rhs=rec_b[0:97, :],
                             start=True, stop=True)
            nc.vector.tensor_mul(
                attnT[p][:, sq * 512:(sq + 1) * 512], pv_sb[:], rbp[:])

        def flush_norm():
            while norm_pending:
                do_norm(*norm_pending.pop(0))

        for sq in range(SQB):
            qs = slice(sq * 512, (sq + 1) * 512)
            dn = ps_dn.tile([128, 512], f32, tag="mm", name="dn")
            pv_sbs = []
            for p in range(NPAIR):
                xps = ps_pv.tile([128, 512], f32, tag="mm", name="xps")
                for sk in range(SKT):
                    sc = ps_sc.tile([128, 1024], f32, tag="sc", name="sc")
                    for h in range(2):
                        hp = h * 64
                        nc.tensor.matmul(
                            sc[:, h * 512:(h + 1) * 512],
                            lhsT=kTt[p][hp:hp + 64, sk * 128:(sk + 1) * 128],
                            rhs=qT[p][hp:hp + 64, qs],
                            start=True, stop=True,
                        )
                    pt = ptpool.tile([128, 1024], bf16, tag="pt")
                    nc.scalar.activation(pt[:], sc[:], Act.Exp, scale=0.125)
                    for h in range(2):
                        nc.tensor.matmul(
                            xps[h * 64:(h + 1) * 64, :],
                            lhsT=v_t[sk][:, p * 128 + h * 64:p * 128 + (h + 1) * 64],
                            rhs=pt[:, h * 512:(h + 1) * 512],
                            start=(sk == 0), stop=(sk == SKT - 1),
                        )
                    for h in range(2):
                        r = p * 64 + h * 32
                        nc.tensor.matmul(
                            dn[r:r + 1, :], lhsT=ones_col[:, 0:1],
                            rhs=pt[:, h * 512:(h + 1) * 512],
                            start=(sk == 0), stop=(sk == SKT - 1),
                            tile_position=(0, r),
                        )
                    # deferred work from the previous sq, placed where its
                    # inputs are long since ready so the PE never stalls
                    if sk == 4:
                        flush_norm()
                    if sk >= 8 and wo_pending:
                        emit_wo([wo_pending.pop(0)], ps_sh, "sh")
                # copy PV out of psum promptly so the next sq can reuse it
                pv_sb = pvsb.tile([128, 512], f32, tag="pvsb", name="pvsb")
                nc.vector.tensor_copy(pv_sb[:], xps[:])
                pv_sbs.append(pv_sb)

            # denominators -> batched fast reciprocal (both pairs at once)
            den_sb = smpool.tile([97, 512], f32, tag="densb", name="densb")
            nc.vector.tensor_copy(den_sb[:], dn[0:97, :])
            rec_f = smpool.tile([97, 512], f32, tag="recf", name="recf")
            nc.vector.reciprocal_approx_fast(rec_f[:], den_sb[:])
            rec_b = smpool.tile([97, 512], bf16, tag="recb", name="recb")
            nc.vector.tensor_copy(rec_b[:], rec_f[:])
            for p in range(NPAIR):
                norm_pending.append((p, sq, pv_sbs[p], rec_b))

            wo_pending.extend(range(sq * 4, (sq + 1) * 4))

        # drain: last sq's normalization + remaining Wo via the (now idle)
        # scores pool for 2-deep pipelining
        flush_norm()
        emit_wo(wo_pending, ps_sc, "sc")

    nc.compile()
    return nc


def _get_nc():
    if "nc" not in _cache:
        _cache["nc"] = _build()
    return _cache["nc"]


def kernel(query, key, value, Wq, bq, Wk, bk, Wv, bv, Wo, bo, trace=False):
    import ml_dtypes
    from concourse.bass_utils import run_bass_kernel_spmd

    nc = _get_nc()
    bf = ml_dtypes.bfloat16

    q = np.asarray(query, np.float32)
    k = np.asarray(key, np.float32)
    v = np.asarray(value, np.float32)
    xT = {}
    for nm, x in (("q", q), ("k", k), ("v", v)):
        for b in range(B):
            xT[(nm, b)] = np.ascontiguousarray(x[b].T).astype(bf)
    Wq = np.asarray(Wq, np.float32).astype(bf)
    Wk = np.asarray(Wk, np.float32).astype(bf)
    Wv = np.asarray(Wv, np.float32).astype(bf)
    Wo = np.asarray(Wo, np.float32).astype(bf)
    bqf = np.asarray(bq, np.float32)
    bkf = np.asarray(bk, np.float32)
    bvf = np.asarray(bv, np.float32).astype(bf)

    in_maps = []
    for r in range(NCORES):
        b, g = divmod(r, GC)
        sl = slice(g * DKC, (g + 1) * DKC)
        in_maps.append({
            "xqT": xT[("q", b)], "xkT": xT[("k", b)], "xvT": xT[("v", b)],
            "wq": np.ascontiguousarray(Wq[:, sl]),
            "wk": np.ascontiguousarray(Wk[:, sl]),
            "wv": np.ascontiguousarray(Wv[:, sl]),
            "wo": np.ascontiguousarray(Wo[sl, :]),
            "bq": np.ascontiguousarray(bqf[sl, None]),
            "bk": np.ascontiguousarray(bkf[sl, None]),
            "bv": np.ascontiguousarray(bvf[None, sl]),
        })

    res = run_bass_kernel_spmd(nc, in_maps, list(range(NCORES)), trace=trace)
    _cache["last_results"] = res

    bo = np.asarray(bo, np.float32)
    out = np.empty((B, S, D), np.float32)
    for b in range(B):
        acc = np.zeros((S, D), np.float32)
        for g in range(GC):
            acc += np.asarray(res.results[b * GC + g]["out"], np.float32)
        out[b] = acc + bo[None, :]
    return out


# revision 10
# speedup vs baseline: 2.8699x; 1.0837x over previous
"""Multi-head attention (B=2, S=2048, D=1024, H=16) on 8 NeuronCores.

Sharding: DP(batch) x TP(heads). Core r handles batch r//4 and heads
[4*(r%4), 4*(r%4)+4) as two head-pairs. Per core:
  - q/k/v projections for its batch tokens x its 256 dk columns
  - attention for its 4 heads (head-pair packed matmuls:
      scores: two K=64 matmuls row-packed at partitions 0/64
      PV:     two M=64 matmuls col-packed at psum rows 0/64
      denom:  M=1 ones-matmuls col-packed at psum rows 0/32/64/96)
  - Wo row-slice partial [2048, 1024]
No collectives: the host sums the 4 partials per batch and adds bo.

All matmul operands are bf16 (fp32 PSUM accumulate). exp on ACT as one
[128,1024] instruction spanning 2 psum banks per (pair, sq, sk).
Softmax 1/den via reciprocal_approx_fast batched [97,512].
"""

import sys

sys.path.insert(0, "/opt/trn_rl_repo")

import numpy as np

B, S, D, H, DK = 2, 2048, 1024, 16, 64
NCORES = 8
GC = 4                 # cores per batch group
HPC = 4                # heads per core
DKC = HPC * DK         # 256 dk columns per core
NPAIR = 2              # head pairs per core
KT = D // 128          # 8 contraction tiles for projections
SQB = S // 512         # 4 query blocks
SKT = S // 128         # 16 key tiles

_cache = {}


def _build():
    from contextlib import ExitStack

    from concourse import bacc
    import concourse.mybir as mybir
    import concourse.tile as tile

    f32 = mybir.dt.float32
    bf16 = mybir.dt.bfloat16
    Act = mybir.ActivationFunctionType

    nc = bacc.Bacc(
        "TRN2", target_bir_lowering=False, debug=False,
        enable_asserts=False, num_devices=NCORES,
    )

    xqT = nc.dram_tensor("xqT", [D, S], bf16, kind="ExternalInput").ap()
    xkT = nc.dram_tensor("xkT", [D, S], bf16, kind="ExternalInput").ap()
    xvT = nc.dram_tensor("xvT", [D, S], bf16, kind="ExternalInput").ap()
    wq = nc.dram_tensor("wq", [D, DKC], bf16, kind="ExternalInput").ap()
    wk = nc.dram_tensor("wk", [D, DKC], bf16, kind="ExternalInput").ap()
    wv = nc.dram_tensor("wv", [D, DKC], bf16, kind="ExternalInput").ap()
    wo = nc.dram_tensor("wo", [DKC, D], bf16, kind="ExternalInput").ap()
    bq = nc.dram_tensor("bq", [DKC, 1], f32, kind="ExternalInput").ap()
    bk = nc.dram_tensor("bk", [DKC, 1], f32, kind="ExternalInput").ap()
    bv = nc.dram_tensor("bv", [1, DKC], bf16, kind="ExternalInput").ap()
    out_ext = nc.dram_tensor("out", [S, D], bf16, kind="ExternalOutput").ap()

    with tile.TileContext(nc) as tc, ExitStack() as ctx, \
            nc.allow_low_precision("bf16 matmul operands, fp32 psum accumulate"):
        wpool = ctx.enter_context(tc.tile_pool(name="w", bufs=1))
        xpool = ctx.enter_context(tc.tile_pool(name="x", bufs=24))
        qkpool = ctx.enter_context(tc.tile_pool(name="qk", bufs=1))
        vpool = ctx.enter_context(tc.tile_pool(name="v", bufs=16))
        ptpool = ctx.enter_context(tc.tile_pool(name="pt", bufs=4))
        atpool = ctx.enter_context(tc.tile_pool(name="at", bufs=1))
        smpool = ctx.enter_context(tc.tile_pool(name="sm", bufs=2))
        pvsb = ctx.enter_context(tc.tile_pool(name="pvsb", bufs=4))
        opool = ctx.enter_context(tc.tile_pool(name="o", bufs=3))
        ps_sc = ctx.enter_context(tc.tile_pool(name="pssc", bufs=2, space="PSUM"))
        ps_pv = ctx.enter_context(tc.tile_pool(name="pspv", bufs=2, space="PSUM"))
        ps_dn = ctx.enter_context(tc.tile_pool(name="psdn", bufs=1, space="PSUM"))
        ps_sh = ctx.enter_context(tc.tile_pool(name="pssh", bufs=1, space="PSUM"))

        # ---- input chunks: one big [128, 2048] DMA per (input, k-chunk),
        # issued first on the sync queue so the first matmul isn't stuck
        # behind weight DMAs. Weights go on the (idle) gpsimd queue.
        xk_t, xq_t, xv_t = [], [], []
        for src, lst in ((xkT, xk_t), (xqT, xq_t), (xvT, xv_t)):
            for k in range(KT):
                t = xpool.tile([128, S], bf16, tag="xt")
                nc.sync.dma_start(t[:], src[k * 128:(k + 1) * 128, :])
                lst.append(t)

        # ---- weights / constants into SBUF ----
        wq_t, wk_t, wv_t = [], [], []
        for name, src, lst in (("wk", wk, wk_t), ("wq", wq, wq_t), ("wv", wv, wv_t)):
            for k in range(KT):
                t = wpool.tile([128, DKC], bf16, tag=f"{name}{k}", name=f"{name}{k}")
                nc.gpsimd.dma_start(t[:], src[k * 128:(k + 1) * 128, :])
                lst.append(t)
        wo_t = []
        for k in range(2):
            t = wpool.tile([128, D], bf16, tag=f"wo{k}", name=f"wo{k}")
            nc.gpsimd.dma_start(t[:], wo[k * 128:(k + 1) * 128, :])
            wo_t.append(t)
        bq_t, bk_t = [], []
        for p in range(NPAIR):
            t = wpool.tile([128, 1], f32, tag=f"bq{p}", name=f"bq{p}")
            nc.gpsimd.dma_start(t[:], bq[p * 128:(p + 1) * 128, :])
            bq_t.append(t)
            t = wpool.tile([128, 1], f32, tag=f"bk{p}", name=f"bk{p}")
            nc.gpsimd.dma_start(t[:], bk[p * 128:(p + 1) * 128, :])
            bk_t.append(t)
        bv_t = wpool.tile([1, DKC], bf16, tag="bv")
        nc.gpsimd.dma_start(bv_t[:], bv[:])

        onesf = wpool.tile([128, 128], f32, tag="onesf")
        nc.gpsimd.memset(onesf[:], 1.0)
        ones_col = wpool.tile([128, 1], bf16, tag="onescol")
        nc.vector.tensor_copy(ones_col[:], onesf[:, 0:1])
        ones_row = wpool.tile([1, 128], bf16, tag="onesrow")
        nc.vector.tensor_copy(ones_row[:], onesf[0:1, :])
        # E97 selectors: rb rows 0:64 <- rec row 64p, rows 64:128 <- row 64p+32
        e97 = []
        for p in range(NPAIR):
            ef = wpool.tile([97, 128], f32, tag=f"e97f{p}", name=f"e97f{p}")
            nc.gpsimd.memset(ef[:], 0.0)
            nc.gpsimd.memset(ef[64 * p:64 * p + 1, 0:64], 1.0)
            nc.gpsimd.memset(ef[64 * p + 32:64 * p + 33, 64:128], 1.0)
            eb = wpool.tile([97, 128], bf16, tag=f"e97_{p}", name=f"e97_{p}")
            nc.vector.tensor_copy(eb[:], ef[:])
            e97.append(eb)

        # ---- k/q projections -> kT_p, qT_p [128, S] bf16 ----
        qT = [qkpool.tile([128, S], bf16, tag=f"qT{p}", name=f"qT{p}")
              for p in range(NPAIR)]
        kTt = [qkpool.tile([128, S], bf16, tag=f"kT{p}", name=f"kT{p}")
               for p in range(NPAIR)]
        for xts, w_list, b_list, dsts in (
            (xk_t, wk_t, bk_t, kTt), (xq_t, wq_t, bq_t, qT),
        ):
            for blk in range(SQB):
                bs = slice(blk * 512, (blk + 1) * 512)
                for p, pspool in ((0, ps_pv), (1, ps_dn)):
                    ps = pspool.tile([128, 512], f32, tag="mm", name="projps")
                    for k in range(KT):
                        nc.tensor.matmul(
                            ps[:], lhsT=w_list[k][:, p * 128:(p + 1) * 128],
                            rhs=xts[k][:, bs], start=(k == 0), stop=(k == KT - 1),
                        )
                    nc.scalar.activation(
                        dsts[p][:, bs], ps[:],
                        Act.Identity, bias=b_list[p][:, 0:1],
                    )

        # ---- v projection -> 16 tiles [128 tok, 256] bf16 ----
        v_t = []
        for tt in range(SKT):
            pspool, pstag = ((ps_pv, "mm"), (ps_dn, "mm"),
                             (ps_sh, "sh"), (ps_pv, "mm"))[tt % 4]
            ps = pspool.tile([128, DKC], f32, tag=pstag, name="vps")
            for k in range(KT):
                nc.tensor.matmul(
                    ps[:, 0:DKC], lhsT=xv_t[k][:, tt * 128:(tt + 1) * 128],
                    rhs=wv_t[k][:], start=(k == 0), stop=False,
                )
            nc.tensor.matmul(
                ps[:, 0:DKC], lhsT=ones_row[0:1, :], rhs=bv_t[:],
                start=False, stop=True,
            )
            vt = vpool.tile([128, DKC], bf16, tag="v")
            nc.vector.tensor_copy(vt[:], ps[:, 0:DKC])
            v_t.append(vt)

        # ---- attention + Wo ----
        attnT = [atpool.tile([128, S], bf16, tag=f"attnT{p}", name=f"attnT{p}")
                 for p in range(NPAIR)]
        wo_pending = []    # (m, n) groups whose attnT deps are satisfied
        norm_pending = []  # deferred normalization closures

        def emit_wo(ms, pspool, pstag):
            for m in ms:
                ot = opool.tile([128, D], bf16, tag="ot", name="ot")
                for n in range(2):
                    ps = pspool.tile([128, 512], f32, tag=pstag, name="wops")
                    for p in range(NPAIR):
                        nc.tensor.matmul(
                            ps[:], lhsT=attnT[p][:, m * 128:(m + 1) * 128],
                            rhs=wo_t[p][:, n * 512:(n + 1) * 512],
                            start=(p == 0), stop=(p == NPAIR - 1),
                        )
                    nc.vector.tensor_copy(ot[:, n * 512:(n + 1) * 512], ps[:])
                nc.sync.dma_start(out_ext[m * 128:(m + 1) * 128, :], ot[:])

        def do_norm(p, sq, pv_sb, rec_b):
            # rb = broadcast of rec rows (64p, 64p+32) over dk rows; then
            # attnT[:, sq block] = pv_sb * rb
            rbp = ps_sh.tile([128, 512], f32, tag="sh", name="rbp")
            nc.tensor.matmul(rbp[:], lhsT=e97[p][:], rhs=rec_b[0:97, :],
                             start=True, stop=True)
            nc.vector.tensor_mul(
                attnT[p][:, sq * 512:(sq + 1) * 512], pv_sb[:], rbp[:])

        def flush_norm():
            while norm_pending:
                do_norm(*norm_pending.pop(0))

        for sq in range(SQB):
            qs = slice(sq * 512, (sq + 1) * 512)
            dn = ps_dn.tile([128, 512], f32, tag="mm", name="dn")
            pv_sbs = []
            for p in range(NPAIR):
                xps = ps_pv.tile([128, 512], f32, tag="mm", name="xps")

                # scores for step sk, emitted 2 steps ahead of their
                # consumers so the ACT exp stream never starves the PE
                # (and vice versa)
                sc_tiles = {}

                def emit_scores(sk):
                    sc = ps_sc.tile([128, 1024], f32, tag="sc", name="sc")
                    for h in range(2):
                        hp = h * 64
                        nc.tensor.matmul(
                            sc[:, h * 512:(h + 1) * 512],
                            lhsT=kTt[p][hp:hp + 64, sk * 128:(sk + 1) * 128],
                            rhs=qT[p][hp:hp + 64, qs],
                            start=True, stop=True,
                        )
                    sc_tiles[sk] = sc

                emit_scores(0)
                emit_scores(1)
                for sk in range(SKT):
                    sc = sc_tiles.pop(sk)
                    pt = ptpool.tile([128, 1024], bf16, tag="pt")
                    nc.scalar.activation(pt[:], sc[:], Act.Exp, scale=0.125)
                    if sk + 2 < SKT:
                        emit_scores(sk + 2)
                    for h in range(2):
                        nc.tensor.matmul(
                            xps[h * 64:(h + 1) * 64, :],
                            lhsT=v_t[sk][:, p * 128 + h * 64:p * 128 + (h + 1) * 64],
                            rhs=pt[:, h * 512:(h + 1) * 512],
                            start=(sk == 0), stop=(sk == SKT - 1),
                        )
                    for h in range(2):
                        r = p * 64 + h * 32
                        nc.tensor.matmul(
                            dn[r:r + 1, :], lhsT=ones_col[:, 0:1],
                            rhs=pt[:, h * 512:(h + 1) * 512],
                            start=(sk == 0), stop=(sk == SKT - 1),
                            tile_position=(0, r),
                        )
                    # deferred work from the previous sq, placed where its
                    # inputs are long since ready so the PE never stalls
                    if sk == 4:
                        flush_norm()
                    if sk >= 8 and wo_pending:
                        emit_wo([wo_pending.pop(0)], ps_sh, "sh")
                # copy PV out of psum promptly so the next sq can reuse it
                pv_sb = pvsb.tile([128, 512], f32, tag="pvsb", name="pvsb")
                nc.vector.tensor_copy(pv_sb[:], xps[:])
                pv_sbs.append(pv_sb)

            # denominators -> batched fast reciprocal (both pairs at once)
            den_sb = smpool.tile([97, 512], f32, tag="densb", name="densb")
            nc.vector.tensor_copy(den_sb[:], dn[0:97, :])
            rec_f = smpool.tile([97, 512], f32, tag="recf", name="recf")
            nc.vector.reciprocal_approx_fast(rec_f[:], den_sb[:])
            rec_b = smpool.tile([97, 512], bf16, tag="recb", name="recb")
            nc.vector.tensor_copy(rec_b[:], rec_f[:])
            for p in range(NPAIR):
                norm_pending.append((p, sq, pv_sbs[p], rec_b))

            wo_pending.extend(range(sq * 4, (sq + 1) * 4))

        # drain: last sq's normalization + remaining Wo via the (now idle)
        # scores pool for 2-deep pipelining
        flush_norm()
        emit_wo(wo_pending, ps_sc, "sc")

    nc.compile()
    return nc


def _get_nc():
    if "nc" not in _cache:
        _cache["nc"] = _build()
    return _cache["nc"]


def kernel(query, key, value, Wq, bq, Wk, bk, Wv, bv, Wo, bo, trace=False):
    import ml_dtypes
    from concourse.bass_utils import run_bass_kernel_spmd

    nc = _get_nc()
    bf = ml_dtypes.bfloat16

    q = np.asarray(query, np.float32)
    k = np.asarray(key, np.float32)
    v = np.asarray(value, np.float32)
    xT = {}
    for nm, x in (("q", q), ("k", k), ("v", v)):
        for b in range(B):
            xT[(nm, b)] = np.ascontiguousarray(x[b].T).astype(bf)
    Wq = np.asarray(Wq, np.float32).astype(bf)
    Wk = np.asarray(Wk, np.float32).astype(bf)
    Wv = np.asarray(Wv, np.float32).astype(bf)
    Wo = np.asarray(Wo, np.float32).astype(bf)
    bqf = np.asarray(bq, np.float32)
    bkf = np.asarray(bk, np.float32)
    bvf = np.asarray(bv, np.float32).astype(bf)

    in_maps = []
    for r in range(NCORES):
        b, g = divmod(r, GC)
        sl = slice(g * DKC, (g + 1) * DKC)
        in_maps.append({
            "xqT": xT[("q", b)], "xkT": xT[("k", b)], "xvT": xT[("v", b)],
            "wq": np.ascontiguousarray(Wq[:, sl]),
            "wk": np.ascontiguousarray(Wk[:, sl]),
            "wv": np.ascontiguousarray(Wv[:, sl]),
            "wo": np.ascontiguousarray(Wo[sl, :]),
            "bq": np.ascontiguousarray(bqf[sl, None]),
            "bk": np.ascontiguousarray(bkf[sl, None]),
            "bv": np.ascontiguousarray(bvf[None, sl]),
        })

    res = run_bass_kernel_spmd(nc, in_maps, list(range(NCORES)), trace=trace)
    _cache["last_results"] = res

    bo = np.asarray(bo, np.float32)
    out = np.empty((B, S, D), np.float32)
    for b in range(B):
        acc = np.zeros((S, D), np.float32)
        for g in range(GC):
            acc += np.asarray(res.results[b * GC + g]["out"], np.float32)
        out[b] = acc + bo[None, :]
    return out


# revision 11
# speedup vs baseline: 3.3874x; 1.1803x over previous
"""Multi-head attention (B=2, S=2048, D=1024, H=16) on 8 NeuronCores.

Sharding: DP(batch) x TP(heads). Core r handles batch r//4 and heads
[4*(r%4), 4*(r%4)+4) as two head-pairs. Per core:
  - q/k/v projections for its batch tokens x its 256 dk columns
  - attention for its 4 heads (head-pair packed matmuls:
      scores: two K=64 matmuls row-packed at partitions 0/64
      PV:     two M=64 matmuls col-packed at psum rows 0/64
      denom:  M=1 ones-matmuls col-packed at psum rows 0/32/64/96)
  - Wo row-slice partial [2048, 1024]
No collectives: the host sums the 4 partials per batch and adds bo.

All matmul operands are bf16 (fp32 PSUM accumulate). exp on ACT as one
[128,1024] instruction spanning 2 psum banks per (pair, sq, sk).
Softmax 1/den via reciprocal_approx_fast batched [97,512].
"""

import sys

sys.path.insert(0, "/opt/trn_rl_repo")

import numpy as np

B, S, D, H, DK = 2, 2048, 1024, 16, 64
NCORES = 8
GC = 4                 # cores per batch group
HPC = 4                # heads per core
DKC = HPC * DK         # 256 dk columns per core
NPAIR = 2              # head pairs per core
KT = D // 128          # 8 contraction tiles for projections
SQB = S // 512         # 4 query blocks
SKT = S // 128         # 16 key tiles

_cache = {}


def _build():
    from contextlib import ExitStack

    from concourse import bacc
    import concourse.mybir as mybir
    import concourse.tile as tile

    f32 = mybir.dt.float32
    bf16 = mybir.dt.bfloat16
    Act = mybir.ActivationFunctionType

    nc = bacc.Bacc(
        "TRN2", target_bir_lowering=False, debug=False,
        enable_asserts=False, num_devices=NCORES,
    )

    xqT = nc.dram_tensor("xqT", [D, S], bf16, kind="ExternalInput").ap()
    xkT = nc.dram_tensor("xkT", [D, S], bf16, kind="ExternalInput").ap()
    xvT = nc.dram_tensor("xvT", [D, S], bf16, kind="ExternalInput").ap()
    wq = nc.dram_tensor("wq", [D, DKC], bf16, kind="ExternalInput").ap()
    wk = nc.dram_tensor("wk", [D, DKC], bf16, kind="ExternalInput").ap()
    wv = nc.dram_tensor("wv", [D, DKC], bf16, kind="ExternalInput").ap()
    wo = nc.dram_tensor("wo", [DKC, D], bf16, kind="ExternalInput").ap()
    bq = nc.dram_tensor("bq", [DKC, 1], f32, kind="ExternalInput").ap()
    bk = nc.dram_tensor("bk", [DKC, 1], f32, kind="ExternalInput").ap()
    bv = nc.dram_tensor("bv", [1, DKC], bf16, kind="ExternalInput").ap()
    out_ext = nc.dram_tensor("out", [S, D], bf16, kind="ExternalOutput").ap()

    with tile.TileContext(nc) as tc, ExitStack() as ctx, \
            nc.allow_low_precision("bf16 matmul operands, fp32 psum accumulate"):
        wpool = ctx.enter_context(tc.tile_pool(name="w", bufs=1))
        xpool = ctx.enter_context(tc.tile_pool(name="x", bufs=24))
        qkpool = ctx.enter_context(tc.tile_pool(name="qk", bufs=1))
        vpool = ctx.enter_context(tc.tile_pool(name="v", bufs=16))
        ptpool = ctx.enter_context(tc.tile_pool(name="pt", bufs=4))
        atpool = ctx.enter_context(tc.tile_pool(name="at", bufs=1))
        smpool = ctx.enter_context(tc.tile_pool(name="sm", bufs=2))
        pvsb = ctx.enter_context(tc.tile_pool(name="pvsb", bufs=4))
        opool = ctx.enter_context(tc.tile_pool(name="o", bufs=3))
        ps_sc = ctx.enter_context(tc.tile_pool(name="pssc", bufs=2, space="PSUM"))
        ps_pv = ctx.enter_context(tc.tile_pool(name="pspv", bufs=2, space="PSUM"))
        ps_dn = ctx.enter_context(tc.tile_pool(name="psdn", bufs=1, space="PSUM"))
        ps_sh = ctx.enter_context(tc.tile_pool(name="pssh", bufs=1, space="PSUM"))

        # ---- input chunks: one big [128, 2048] DMA per (input, k-chunk),
        # issued first on the sync queue so the first matmul isn't stuck
        # behind weight DMAs. Weights go on the (idle) gpsimd queue.
        xk_t, xq_t, xv_t = [], [], []
        for src, lst in ((xkT, xk_t), (xqT, xq_t), (xvT, xv_t)):
            for k in range(KT):
                t = xpool.tile([128, S], bf16, tag="xt")
                nc.sync.dma_start(t[:], src[k * 128:(k + 1) * 128, :])
                lst.append(t)

        # ---- weights / constants into SBUF ----
        wq_t, wk_t, wv_t = [], [], []
        for name, src, lst in (("wk", wk, wk_t), ("wq", wq, wq_t), ("wv", wv, wv_t)):
            for k in range(KT):
                t = wpool.tile([128, DKC], bf16, tag=f"{name}{k}", name=f"{name}{k}")
                nc.gpsimd.dma_start(t[:], src[k * 128:(k + 1) * 128, :])
                lst.append(t)
        wo_t = []
        for k in range(2):
            t = wpool.tile([128, D], bf16, tag=f"wo{k}", name=f"wo{k}")
            nc.gpsimd.dma_start(t[:], wo[k * 128:(k + 1) * 128, :])
            wo_t.append(t)
        bq_t, bk_t = [], []
        for p in range(NPAIR):
            t = wpool.tile([128, 1], f32, tag=f"bq{p}", name=f"bq{p}")
            nc.gpsimd.dma_start(t[:], bq[p * 128:(p + 1) * 128, :])
            bq_t.append(t)
            t = wpool.tile([128, 1], f32, tag=f"bk{p}", name=f"bk{p}")
            nc.gpsimd.dma_start(t[:], bk[p * 128:(p + 1) * 128, :])
            bk_t.append(t)
        bv_t = wpool.tile([1, DKC], bf16, tag="bv")
        nc.gpsimd.dma_start(bv_t[:], bv[:])

        onesf = wpool.tile([128, 128], f32, tag="onesf")
        nc.gpsimd.memset(onesf[:], 1.0)
        ones_col = wpool.tile([128, 1], bf16, tag="onescol")
        nc.vector.tensor_copy(ones_col[:], onesf[:, 0:1])
        ones_row = wpool.tile([1, 128], bf16, tag="onesrow")
        nc.vector.tensor_copy(ones_row[:], onesf[0:1, :])
        # E97 selectors: rb rows 0:64 <- rec row 64p, rows 64:128 <- row 64p+32
        e97 = []
        for p in range(NPAIR):
            ef = wpool.tile([97, 128], f32, tag=f"e97f{p}", name=f"e97f{p}")
            nc.gpsimd.memset(ef[:], 0.0)
            nc.gpsimd.memset(ef[64 * p:64 * p + 1, 0:64], 1.0)
            nc.gpsimd.memset(ef[64 * p + 32:64 * p + 33, 64:128], 1.0)
            eb = wpool.tile([97, 128], bf16, tag=f"e97_{p}", name=f"e97_{p}")
            nc.vector.tensor_copy(eb[:], ef[:])
            e97.append(eb)

        # ---- k/q projections -> kT_p, qT_p [128, S] bf16 ----
        qT = [qkpool.tile([128, S], bf16, tag=f"qT{p}", name=f"qT{p}")
              for p in range(NPAIR)]
        kTt = [qkpool.tile([128, S], bf16, tag=f"kT{p}", name=f"kT{p}")
               for p in range(NPAIR)]
        # k-outer / blk-inner: 4 consecutive matmuls share one stationary
        # weight tile, and the first matmul only needs the first x chunk
        for xts, w_list, b_list, dsts in (
            (xk_t, wk_t, bk_t, kTt), (xq_t, wq_t, bq_t, qT),
        ):
            for p in range(NPAIR):
                pss = [(ps_pv, "mm"), (ps_dn, "mm"), (ps_sh, "sh"),
                       (ps_pv, "mm")]
                pstiles = [pool.tile([128, 512], f32, tag=tag, name="projps")
                           for pool, tag in pss]
                for k in range(KT):
                    for blk in range(SQB):
                        nc.tensor.matmul(
                            pstiles[blk][:],
                            lhsT=w_list[k][:, p * 128:(p + 1) * 128],
                            rhs=xts[k][:, blk * 512:(blk + 1) * 512],
                            start=(k == 0), stop=(k == KT - 1),
                        )
                for blk in range(SQB):
                    nc.scalar.activation(
                        dsts[p][:, blk * 512:(blk + 1) * 512], pstiles[blk][:],
                        Act.Identity, bias=b_list[p][:, 0:1],
                    )

        # ---- v projection -> 16 tiles [128 tok, 256] bf16 ----
        v_t = []
        for tt in range(SKT):
            pspool, pstag = ((ps_pv, "mm"), (ps_dn, "mm"),
                             (ps_sh, "sh"), (ps_pv, "mm"))[tt % 4]
            ps = pspool.tile([128, DKC], f32, tag=pstag, name="vps")
            for k in range(KT):
                nc.tensor.matmul(
                    ps[:, 0:DKC], lhsT=xv_t[k][:, tt * 128:(tt + 1) * 128],
                    rhs=wv_t[k][:], start=(k == 0), stop=False,
                )
            nc.tensor.matmul(
                ps[:, 0:DKC], lhsT=ones_row[0:1, :], rhs=bv_t[:],
                start=False, stop=True,
            )
            vt = vpool.tile([128, DKC], bf16, tag="v")
            nc.vector.tensor_copy(vt[:], ps[:, 0:DKC])
            v_t.append(vt)

        # ---- attention + Wo ----
        attnT = [atpool.tile([128, S], bf16, tag=f"attnT{p}", name=f"attnT{p}")
                 for p in range(NPAIR)]
        wo_pending = []    # (m, n) groups whose attnT deps are satisfied
        norm_pending = []  # deferred normalization closures

        def emit_wo(ms, pspool, pstag):
            for m in ms:
                ot = opool.tile([128, D], bf16, tag="ot", name="ot")
                for n in range(2):
                    ps = pspool.tile([128, 512], f32, tag=pstag, name="wops")
                    for p in range(NPAIR):
                        nc.tensor.matmul(
                            ps[:], lhsT=attnT[p][:, m * 128:(m + 1) * 128],
                            rhs=wo_t[p][:, n * 512:(n + 1) * 512],
                            start=(p == 0), stop=(p == NPAIR - 1),
                        )
                    nc.vector.tensor_copy(ot[:, n * 512:(n + 1) * 512], ps[:])
                nc.sync.dma_start(out_ext[m * 128:(m + 1) * 128, :], ot[:])

        def do_norm(p, sq, pv_sb, rec_b):
            # rb = broadcast of rec rows (64p, 64p+32) over dk rows; then
            # attnT[:, sq block] = pv_sb * rb
            rbp = ps_sh.tile([128, 512], f32, tag="sh", name="rbp")
            nc.tensor.matmul(rbp[:], lhsT=e97[p][:], rhs=rec_b[0:97, :],
                             start=True, stop=True)
            nc.vector.tensor_mul(
                attnT[p][:, sq * 512:(sq + 1) * 512], pv_sb[:], rbp[:])

        def flush_norm():
            while norm_pending:
                do_norm(*norm_pending.pop(0))

        for sq in range(SQB):
            qs = slice(sq * 512, (sq + 1) * 512)
            dn = ps_dn.tile([128, 512], f32, tag="mm", name="dn")
            pv_sbs = []
            for p in range(NPAIR):
                xps = ps_pv.tile([128, 512], f32, tag="mm", name="xps")

                # scores for step sk, emitted 2 steps ahead of their
                # consumers so the ACT exp stream never starves the PE
                # (and vice versa)
                sc_tiles = {}

                def emit_scores(sk):
                    sc = ps_sc.tile([128, 1024], f32, tag="sc", name="sc")
                    for h in range(2):
                        hp = h * 64
                        nc.tensor.matmul(
                            sc[:, h * 512:(h + 1) * 512],
                            lhsT=kTt[p][hp:hp + 64, sk * 128:(sk + 1) * 128],
                            rhs=qT[p][hp:hp + 64, qs],
                            start=True, stop=True,
                        )
                    sc_tiles[sk] = sc

                emit_scores(0)
                emit_scores(1)
                for sk in range(SKT):
                    sc = sc_tiles.pop(sk)
                    pt = ptpool.tile([128, 1024], bf16, tag="pt")
                    nc.scalar.activation(pt[:], sc[:], Act.Exp, scale=0.125)
                    if sk + 2 < SKT:
                        emit_scores(sk + 2)
                    for h in range(2):
                        nc.tensor.matmul(
                            xps[h * 64:(h + 1) * 64, :],
                            lhsT=v_t[sk][:, p * 128 + h * 64:p * 128 + (h + 1) * 64],
                            rhs=pt[:, h * 512:(h + 1) * 512],
                            start=(sk == 0), stop=(sk == SKT - 1),
                        )
                    for h in range(2):
                        r = p * 64 + h * 32
                        nc.tensor.matmul(
                            dn[r:r + 1, :], lhsT=ones_col[:, 0:1],
                            rhs=pt[:, h * 512:(h + 1) * 512],
                            start=(sk == 0), stop=(sk == SKT - 1),
                            tile_position=(0, r),
                        )
                    # deferred work from the previous sq, placed where its
                    # inputs are long since ready so the PE never stalls
                    if sk == 4:
                        flush_norm()
                    if sk >= 8 and wo_pending:
                        emit_wo([wo_pending.pop(0)], ps_sh, "sh")
                # copy PV out of psum promptly so the next sq can reuse it
                pv_sb = pvsb.tile([128, 512], f32, tag="pvsb", name="pvsb")
                nc.vector.tensor_copy(pv_sb[:], xps[:])
                pv_sbs.append(pv_sb)

            # denominators -> batched fast reciprocal (both pairs at once)
            den_sb = smpool.tile([97, 512], f32, tag="densb", name="densb")
            nc.vector.tensor_copy(den_sb[:], dn[0:97, :])
            rec_f = smpool.tile([97, 512], f32, tag="recf", name="recf")
            nc.vector.reciprocal_approx_fast(rec_f[:], den_sb[:])
            rec_b = smpool.tile([97, 512], bf16, tag="recb", name="recb")
            nc.vector.tensor_copy(rec_b[:], rec_f[:])
            for p in range(NPAIR):
                norm_pending.append((p, sq, pv_sbs[p], rec_b))

            wo_pending.extend(range(sq * 4, (sq + 1) * 4))

        # drain: last sq's normalization + remaining Wo via the (now idle)
        # scores pool for 2-deep pipelining
        flush_norm()
        emit_wo(wo_pending, ps_sc, "sc")

    nc.compile()
    return nc


def _get_nc():
    if "nc" not in _cache:
        _cache["nc"] = _build()
    return _cache["nc"]


def kernel(query, key, value, Wq, bq, Wk, bk, Wv, bv, Wo, bo, trace=False):
    import ml_dtypes
    from concourse.bass_utils import run_bass_kernel_spmd

    nc = _get_nc()
    bf = ml_dtypes.bfloat16

    q = np.asarray(query, np.float32)
    k = np.asarray(key, np.float32)
    v = np.asarray(value, np.float32)
    xT = {}
    for nm, x in (("q", q), ("k", k), ("v", v)):
        for b in range(B):
            xT[(nm, b)] = np.ascontiguousarray(x[b].T).astype(bf)
    Wq = np.asarray(Wq, np.float32).astype(bf)
    Wk = np.asarray(Wk, np.float32).astype(bf)
    Wv = np.asarray(Wv, np.float32).astype(bf)
    Wo = np.asarray(Wo, np.float32).astype(bf)
    bqf = np.asarray(bq, np.float32)
    bkf = np.asarray(bk, np.float32)
    bvf = np.asarray(bv, np.float32).astype(bf)

    in_maps = []
    for r in range(NCORES):
        b, g = divmod(r, GC)
        sl = slice(g * DKC, (g + 1) * DKC)
        in_maps.append({
            "xqT": xT[("q", b)], "xkT": xT[("k", b)], "xvT": xT[("v", b)],
            "wq": np.ascontiguousarray(Wq[:, sl]),
            "wk": np.ascontiguousarray(Wk[:, sl]),
            "wv": np.ascontiguousarray(Wv[:, sl]),
            "wo": np.ascontiguousarray(Wo[sl, :]),
            "bq": np.ascontiguousarray(bqf[sl, None]),
            "bk": np.ascontiguousarray(bkf[sl, None]),
            "bv": np.ascontiguousarray(bvf[None, sl]),
        })

    res = run_bass_kernel_spmd(nc, in_maps, list(range(NCORES)), trace=trace)
    _cache["last_results"] = res

    bo = np.asarray(bo, np.float32)
    out = np.empty((B, S, D), np.float32)
    for b in range(B):
        acc = np.zeros((S, D), np.float32)
        for g in range(GC):
            acc += np.asarray(res.results[b * GC + g]["out"], np.float32)
        out[b] = acc + bo[None, :]
    return out
